# revision 34
# baseline (speedup 1.0000x reference)
"""Multi-head attention (B=8, N=1024, C=768, H=12) on 8 TRN2 NeuronCores.

Strategy: pure data parallelism over the batch dim — each core computes one
batch element's full attention block. Weights are replicated; no collectives.

Per-core pipeline (all fp32 storage; matmul dtype selectable):
  1. x [1024,768] -> transpose on PE -> xT [768,1024] in SBUF
  2. qkv(q,k):  qkT[feature, tok]   = (qkv_w chunk).T @ xT        (PE)
     qkv(v):    vnat[tok, 65*h+j]   = xT_chunk.T @ qkv_w[:, vcols] (PE)
                (65-col stride per head: 64 v columns + a ones column used
                 later as the softmax-denominator matmul weight)
  3. per head pair (A,B share SBUF partition halves 0:64 / 64:128):
     scoresT[ktok, q] = kT.T @ qT   (two row-tiled concurrent matmuls)
     expT = Exp(scoresT * 0.125)    (ACT, reads PSUM directly)
     AV:   psAV[0:64]   += vA.T  @ expA ; psAV[64:128] += vB.T @ expB
     sums: psSum[0:1]   += ones.T @ expA ; psSum[32:33] += ones.T @ expB
     (col-tiled concurrent matmuls, accumulated over the 8 k-tiles)
     normalize: r = 1/sums (DVE), broadcast across partitions via DMA,
     concatT[64h+hd, tok] = psAV * r   (DVE)
  4. proj: out[tok, c] = concatT_chunk.T @ proj_w + proj_b  (PE + DVE)
"""

import os
import numpy as np

import concourse.bass as bass
import concourse.tile as tile
from concourse import bacc, mybir
from concourse.bass_utils import run_bass_kernel_spmd
from concourse.masks import make_identity

B, N, C, H, HD = 8, 1024, 768, 12, 64
C3 = 3 * C
P = 128
NT = N // P   # 8 token tiles
CK = C // P   # 6 C chunks
QC = 512      # moving-operand chunk (fp32 max 512)
NQ = N // QC  # 2
f32 = mybir.dt.float32
f32r = mybir.dt.float32r

# v pair-block layout: per head pair j the columns are
#   [ vA(0:64) | onesA(64) | onesB(65) | zeros(66:97) | vB(97:161) ]
# lhsT_A = block[0:128]   -> psum rows: 0-63 A-out, 64 A-sums
# lhsT_B = block[33:161]  -> psum rows: 32 B-sums, 64-127 B-out
# Both views are M=128 matmuls with dst partition 0 (required by fp32r),
# and the sums land on 32-aligned psum rows for DVE access.
PW = 161       # pair block width
OFS_B = 33     # lhsT_B offset within the block
VB_OFS = 97    # vB column offset

# matmul operand dtype mode: "fp32" (exact, 4 cyc/row) or "fp32r" (1 cyc/row)
MODE = os.environ.get("ATTN_MM_MODE", "fp32r")


def _mm_dt(mode):
    """Storage dtype for matmul operand tensors. float32r tensors must be
    written by a compute instruction (DVE/ACT) that performs the rounding —
    the BIR verifier enforces this provenance."""
    return f32r if mode == "fp32r" else f32


def _mc(ap, mode):
    # matmul operand dtype now lives on the tensor; kept for call-site compat
    return ap


def build_body(tc, x_d, qkvw_d, qkvb_d, projw_d, projb_d, out_d, mode, dbg=None,
               phases="all"):
    nc = tc.nc
    Act = mybir.ActivationFunctionType

    dm = _mm_dt(mode)
    with tc.tile_pool(name="persist", bufs=1) as persist:
        # ---- persistent tensors ----
        qkT_s = persist.tile([P, 2 * CK, N], dm)        # q,k features x tokens
        vnat_s = persist.tile([P, NT, (H // 2) * PW], dm)  # v pair blocks
        ident = persist.tile([P, P], f32)
        qkvb_qk = persist.tile([P, 2 * CK], f32)
        vb_bc = persist.tile([P, H, HD], f32)
        pb_bc = persist.tile([P, C], f32)

        make_identity(nc, ident)
        nc.sync.dma_start(qkvb_qk, qkvb_d[: 2 * C].rearrange("(m p) -> p m", p=P))
        nc.sync.dma_start(
            vb_bc, qkvb_d[2 * C :].rearrange("(h j) -> h j", j=HD).partition_broadcast(P)
        )
        nc.sync.dma_start(pb_bc, projb_d.partition_broadcast(P))

        # ones + zero filler columns of the v pair blocks (written once).
        # memset cannot write float32r; memset f32 then DVE-copy (rounds).
        vnat_w = vnat_s.rearrange("p t (j w) -> p t j w", w=PW)
        ones_f = persist.tile([P, 1], f32)
        zero_f = persist.tile([P, 1], f32)
        nc.vector.memset(ones_f, 1.0)
        nc.vector.memset(zero_f, 0.0)
        ones_row = persist.tile([P, P], dm)   # all-ones, lhsT of bcast matmuls
        nc.vector.tensor_copy(ones_row, ones_f.to_broadcast([P, P]))
        nc.vector.tensor_copy(
            vnat_w[:, :, :, HD : HD + 2],
            ones_f[:, None, None, :].to_broadcast([P, NT, H // 2, 2]),
        )
        nc.vector.tensor_copy(
            vnat_w[:, :, :, HD + 2 : VB_OFS],
            zero_f[:, None, None, :].to_broadcast([P, NT, H // 2, VB_OFS - HD - 2]),
        )

        # ================= phase A: load + transpose + qkv =================
        with (
            tc.tile_pool(name="phase_a", bufs=1) as pa,
            tc.tile_pool(name="xa", bufs=2) as xa,
            tc.tile_pool(name="pst", bufs=4, space="PSUM") as pst,
            tc.tile_pool(name="mmq", bufs=3, space="PSUM") as mmq,
        ):
            wq_s = pa.tile([P, CK, C3], dm)
            wq_src = qkvw_d.rearrange("(c p) n -> p c n", p=P)
            for c in range(CK):  # chunked: spreads across DMA queues
                nc.sync.dma_start(wq_s[:, c], wq_src[:, c])
            xT_s = pa.tile([P, CK, N], dm)

            x_r = x_d.rearrange("(t p) c -> t p c", p=P)
            for t in range(NT):
                x_t = xa.tile([P, C], f32, tag="xt")
                nc.sync.dma_start(x_t, x_r[t])
                for c in range(CK):
                    pt = pst.tile([P, P], f32, tag="pt")
                    nc.tensor.transpose(pt, x_t[:, c * P : (c + 1) * P], ident)
                    nc.vector.tensor_copy(xT_s[:, c, t * P : (t + 1) * P], pt)

            if phases == "dma":
                # DMA-only bisect: skip all compute, just write something out
                out_r0 = out_d.rearrange("(t p) c -> t p c", p=P)
                for t in range(NT):
                    nc.sync.dma_start(out_r0[t], xT_s[:, :, t * P : (t + 1) * P])
                return

            # q,k features -> qkT  (feature on partitions)
            for m in range(2 * CK):
                for q2 in range(NQ):
                    ps = mmq.tile([P, QC], f32, tag="mm")
                    for k in range(CK):
                        nc.tensor.matmul(
                            ps,
                            lhsT=_mc(wq_s[:, k, m * P : (m + 1) * P], mode),
                            rhs=_mc(xT_s[:, k, q2 * QC : (q2 + 1) * QC], mode),
                            start=(k == 0),
                            stop=(k == CK - 1),
                        )
                    nc.vector.tensor_scalar_add(
                        out=qkT_s[:, m, q2 * QC : (q2 + 1) * QC],
                        in0=ps,
                        scalar1=qkvb_qk[:, m : m + 1],
                    )

            if dbg is not None:
                nc.sync.dma_start(dbg["xT"], xT_s)
                nc.sync.dma_start(dbg["qkT"], qkT_s)

            # v features -> vnat (token on partitions), strided per-head + bias
            for t in range(NT):
                for nv in range(2):
                    nsz = min(QC, C - nv * QC)  # 512, 256
                    h0, nh = nv * 8, nsz // HD
                    ps = mmq.tile([P, QC], f32, tag="mm")
                    for k in range(CK):
                        nc.tensor.matmul(
                            ps[:, :nsz],
                            lhsT=_mc(xT_s[:, k, t * P : (t + 1) * P], mode),
                            rhs=_mc(wq_s[:, k, 2 * C + nv * QC : 2 * C + nv * QC + nsz], mode),
                            start=(k == 0),
                            stop=(k == CK - 1),
                        )
                    pv = ps[:, :nsz].rearrange("p (h j) -> p h j", j=HD)
                    j0 = h0 // 2
                    nc.vector.tensor_add(
                        out=vnat_w[:, t, j0 : j0 + nh // 2, 0:HD],
                        in0=pv[:, 0::2],
                        in1=vb_bc[:, h0 : h0 + nh : 2, :],
                    )
                    nc.vector.tensor_add(
                        out=vnat_w[:, t, j0 : j0 + nh // 2, VB_OFS : VB_OFS + HD],
                        in0=pv[:, 1::2],
                        in1=vb_bc[:, h0 + 1 : h0 + nh : 2, :],
                    )

        if dbg is not None:
            nc.sync.dma_start(dbg["vnat"], vnat_s)

        if phases == "qkv":
            out_r0 = out_d.rearrange("(t p) c -> t p c", p=P)
            for t in range(NT):
                nc.sync.dma_start(out_r0[t], qkT_s[:, 0:CK, t * P : (t + 1) * P])
            return

        # ================= phase B: attention =================
        # separate pool so it reuses the space freed by phase A
        pbc_cm = tc.tile_pool(name="phase_bc", bufs=1)
        pbc = pbc_cm.__enter__()
        concatT_s = pbc.tile([P, CK, N], dm)        # normalized attn out^T
        wp_s = pbc.tile([P, CK, C], dm)
        wp_src = projw_d.rearrange("(c p) n -> p c n", p=P)
        for c in range(CK):
            nc.sync.dma_start(wp_s[:, c], wp_src[:, c])

        with (
            tc.tile_pool(name="exps", bufs=1) as exps,
            tc.tile_pool(name="rpool", bufs=2) as rpool,
            tc.tile_pool(name="sc", bufs=2, space="PSUM") as sc,
            tc.tile_pool(name="avp", bufs=1, space="PSUM") as avp,
        ):
            for j in range(H // 2):
                for q2 in range(NQ):
                    qs = slice(q2 * QC, (q2 + 1) * QC)
                    expA = exps.tile([P, NT, QC], dm, tag="expA")
                    expB = exps.tile([P, NT, QC], dm, tag="expB")
                    # --- scores + exp, two k-tiles per ACT call ---
                    for kp in range(NT // 2):
                        psA = sc.tile([P, 2, QC], f32, tag="sc")
                        psB = sc.tile([P, 2, QC], f32, tag="sc")
                        for u in range(2):
                            kt = 2 * kp + u
                            ks = slice(kt * P, (kt + 1) * P)
                            nc.tensor.matmul(
                                psA[:, u],
                                lhsT=_mc(qkT_s[0:HD, CK + j, ks], mode),
                                rhs=_mc(qkT_s[0:HD, j, qs], mode),
                                start=True, stop=True,
                            )
                            nc.tensor.matmul(
                                psB[:, u],
                                lhsT=_mc(qkT_s[HD:P, CK + j, ks], mode),
                                rhs=_mc(qkT_s[HD:P, j, qs], mode),
                                start=True, stop=True,
                            )
                        nc.scalar.activation(
                            expA[:, 2 * kp : 2 * kp + 2, :], psA, Act.Exp, scale=0.125
                        )
                        nc.scalar.activation(
                            expB[:, 2 * kp : 2 * kp + 2, :], psB, Act.Exp, scale=0.125
                        )
                    if dbg is not None and j == 0 and q2 == 0:
                        nc.sync.dma_start(dbg["expA"], expA)
                        nc.sync.dma_start(dbg["expB"], expB)
                    # --- AV + denominator sums (fused via the pair-block
                    # lhsT views), accumulated over k tiles ---
                    psAV_A = avp.tile([P, QC], f32, tag="avA")
                    psAV_B = avp.tile([P, QC], f32, tag="avB")
                    for kt in range(NT):
                        st, sp = kt == 0, kt == NT - 1
                        lA = vnat_s[:, kt, j * PW : j * PW + P]
                        lB = vnat_s[:, kt, j * PW + OFS_B : j * PW + OFS_B + P]
                        nc.tensor.matmul(
                            psAV_A, lhsT=lA, rhs=_mc(expA[:, kt], mode),
                            start=st, stop=sp,
                        )
                        nc.tensor.matmul(
                            psAV_B, lhsT=lB, rhs=_mc(expB[:, kt], mode),
                            start=st, stop=sp,
                        )
                    # --- normalize into concatT (A sums at psAV_A[64],
                    # B sums at psAV_B[32]) ---
                    # reciprocal (DVE, written as f32r so it can feed a matmul),
                    # then partition-broadcast via PE ones outer product.
                    r_ab = rpool.tile([65, QC], dm, tag="rab")
                    with nc.allow_low_precision(reason="f32r is 4-byte; rounding only"):
                        nc.vector.reciprocal(r_ab[64:65], psAV_A[HD : HD + 1])
                        nc.vector.reciprocal(r_ab[32:33], psAV_B[32:33])
                    psR_A = avp.tile([P, QC], f32, tag="psRA")
                    psR_B = avp.tile([P, QC], f32, tag="psRB")
                    nc.tensor.matmul(
                        psR_A, lhsT=ones_row[HD : HD + 1, :], rhs=r_ab[64:65, :],
                        start=True, stop=True,
                    )
                    nc.tensor.matmul(
                        psR_B, lhsT=ones_row[32:33, :], rhs=r_ab[32:33, :],
                        start=True, stop=True,
                    )
                    rbc = rpool.tile([P, 1, QC], f32, tag="rbc")
                    nc.vector.tensor_copy(rbc[0:HD, 0], psR_A[0:HD])
                    nc.vector.tensor_copy(rbc[HD:P, 0], psR_B[HD:P])
                    nc.vector.tensor_mul(
                        out=concatT_s[0:HD, j, qs], in0=psAV_A[0:HD], in1=rbc[0:HD, 0]
                    )
                    nc.vector.tensor_mul(
                        out=concatT_s[HD:P, j, qs], in0=psAV_B[HD:P], in1=rbc[HD:P, 0]
                    )

        if dbg is not None:
            nc.sync.dma_start(dbg["concatT"], concatT_s)

        if phases == "attn":
            out_r0 = out_d.rearrange("(t p) c -> t p c", p=P)
            for t in range(NT):
                nc.sync.dma_start(out_r0[t], concatT_s[:, :, t * P : (t + 1) * P])
            pbc_cm.__exit__(None, None, None)
            return

        # ================= phase C: output projection =================
        with (
            tc.tile_pool(name="outs", bufs=3) as outs,
            tc.tile_pool(name="mmp", bufs=3, space="PSUM") as mmp,
        ):
            out_r = out_d.rearrange("(t p) c -> t p c", p=P)
            for t in range(NT):
                out_t = outs.tile([P, C], f32, tag="ot")
                for n2 in range(2):
                    nsz = min(QC, C - n2 * QC)
                    ns = slice(n2 * QC, n2 * QC + nsz)
                    ps = mmp.tile([P, QC], f32, tag="mmp")
                    for c in range(CK):
                        nc.tensor.matmul(
                            ps[:, :nsz],
                            lhsT=_mc(concatT_s[:, c, t * P : (t + 1) * P], mode),
                            rhs=_mc(wp_s[:, c, ns], mode),
                            start=(c == 0),
                            stop=(c == CK - 1),
                        )
                    nc.vector.tensor_add(out=out_t[:, ns], in0=ps[:, :nsz], in1=pb_bc[:, ns])
                nc.sync.dma_start(out_r[t], out_t)
        pbc_cm.__exit__(None, None, None)


def build(mode=MODE, repeat=1, debug_dumps=False, phases="all"):
    nc = bacc.Bacc(
        "TRN2",
        target_bir_lowering=False,
        debug=False,
        enable_asserts=False,
        num_devices=B,
    )
    dmw = _mm_dt(mode)
    x_d = nc.dram_tensor("x", [N, C], f32, kind="ExternalInput").ap()
    qkvw_d = nc.dram_tensor("qkv_w", [C, C3], dmw, kind="ExternalInput").ap()
    qkvb_d = nc.dram_tensor("qkv_b", [C3], f32, kind="ExternalInput").ap()
    projw_d = nc.dram_tensor("proj_w", [C, C], dmw, kind="ExternalInput").ap()
    projb_d = nc.dram_tensor("proj_b", [C], f32, kind="ExternalInput").ap()
    out_d = nc.dram_tensor("out", [N, C], f32, kind="ExternalOutput").ap()

    dbg = None
    if debug_dumps:
        dbg = {
            "xT": nc.dram_tensor("dbg_xT", [P, CK, N], f32, kind="ExternalOutput").ap(),
            "qkT": nc.dram_tensor("dbg_qkT", [P, 2 * CK, N], f32, kind="ExternalOutput").ap(),
            "vnat": nc.dram_tensor("dbg_vnat", [P, NT, (H // 2) * PW], f32, kind="ExternalOutput").ap(),
            "expA": nc.dram_tensor("dbg_expA", [P, NT, QC], f32, kind="ExternalOutput").ap(),
            "expB": nc.dram_tensor("dbg_expB", [P, NT, QC], f32, kind="ExternalOutput").ap(),
            "rbc": nc.dram_tensor("dbg_rbc", [P, 1, QC], f32, kind="ExternalOutput").ap(),
            "concatT": nc.dram_tensor("dbg_concatT", [P, CK, N], f32, kind="ExternalOutput").ap(),
        }

    with tile.TileContext(nc) as tc:
        if repeat == 1:
            build_body(tc, x_d, qkvw_d, qkvb_d, projw_d, projb_d, out_d, mode, dbg=dbg, phases=phases)
        else:
            # hardware loop: constant NEFF size, repeat bodies back-to-back --
            # used for timing (wall-clock differencing between repeat counts)
            with tc.For_i(
                0, repeat, 1,
                hint_engines=(mybir.EngineType.PE, mybir.EngineType.DVE),
            ):
                build_body(tc, x_d, qkvw_d, qkvb_d, projw_d, projb_d, out_d, mode, dbg=dbg, phases=phases)
    nc.compile()
    return nc


_NC_CACHE = {}


def _get_nc(mode, repeat=1):
    key = (mode, repeat)
    if key not in _NC_CACHE:
        _NC_CACHE[key] = build(mode, repeat)
    return _NC_CACHE[key]


def kernel(x, qkv_w, qkv_b, proj_w, proj_b):
    x = np.asarray(x, dtype=np.float32)
    qkv_w = np.asarray(qkv_w, dtype=np.float32)
    qkv_b = np.asarray(qkv_b, dtype=np.float32)
    proj_w = np.asarray(proj_w, dtype=np.float32)
    proj_b = np.asarray(proj_b, dtype=np.float32)

    nc = _get_nc(MODE, 1)
    in_maps = [
        {
            "x": np.ascontiguousarray(x[b]),
            "qkv_w": qkv_w,
            "qkv_b": qkv_b,
            "proj_w": proj_w,
            "proj_b": proj_b,
        }
        for b in range(B)
    ]
    res = run_bass_kernel_spmd(nc, in_maps, core_ids=list(range(B)))
    return np.stack([res.results[b]["out"] for b in range(B)]).astype(np.float32)


# revision 37
# speedup vs baseline: 2.1137x; 2.1137x over previous
"""Multi-head attention (B=8, N=1024, C=768, H=12) on 8 TRN2 NeuronCores.

Strategy: pure data parallelism over the batch dim — each core computes one
batch element's full attention block. Weights are replicated; no collectives.

Per-core pipeline (all fp32 storage; matmul dtype selectable):
  1. x [1024,768] -> transpose on PE -> xT [768,1024] in SBUF
  2. qkv(q,k):  qkT[feature, tok]   = (qkv_w chunk).T @ xT        (PE)
     qkv(v):    vnat[tok, 65*h+j]   = xT_chunk.T @ qkv_w[:, vcols] (PE)
                (65-col stride per head: 64 v columns + a ones column used
                 later as the softmax-denominator matmul weight)
  3. per head pair (A,B share SBUF partition halves 0:64 / 64:128):
     scoresT[ktok, q] = kT.T @ qT   (two row-tiled concurrent matmuls)
     expT = Exp(scoresT * 0.125)    (ACT, reads PSUM directly)
     AV:   psAV[0:64]   += vA.T  @ expA ; psAV[64:128] += vB.T @ expB
     sums: psSum[0:1]   += ones.T @ expA ; psSum[32:33] += ones.T @ expB
     (col-tiled concurrent matmuls, accumulated over the 8 k-tiles)
     normalize: r = 1/sums (DVE), broadcast across partitions via DMA,
     concatT[64h+hd, tok] = psAV * r   (DVE)
  4. proj: out[tok, c] = concatT_chunk.T @ proj_w + proj_b  (PE + DVE)
"""

import os
import numpy as np

import concourse.bass as bass
import concourse.tile as tile
from concourse import bacc, mybir
from concourse.bass_utils import run_bass_kernel_spmd
from concourse.masks import make_identity

B, N, C, H, HD = 8, 1024, 768, 12, 64
C3 = 3 * C
P = 128
NT = N // P   # 8 token tiles
CK = C // P   # 6 C chunks
QC = 512      # moving-operand chunk (fp32 max 512)
NQ = N // QC  # 2
f32 = mybir.dt.float32
f32r = mybir.dt.float32r

# v pair-block layout: per head pair j the columns are
#   [ vA(0:64) | onesA(64) | onesB(65) | zeros(66:97) | vB(97:161) ]
# lhsT_A = block[0:128]   -> psum rows: 0-63 A-out, 64 A-sums
# lhsT_B = block[33:161]  -> psum rows: 32 B-sums, 64-127 B-out
# Both views are M=128 matmuls with dst partition 0 (required by fp32r),
# and the sums land on 32-aligned psum rows for DVE access.
PW = 161       # pair block width
OFS_B = 33     # lhsT_B offset within the block
VB_OFS = 97    # vB column offset

# matmul operand dtype mode: "fp32" (exact, 4 cyc/row) or "fp32r" (1 cyc/row)
MODE = os.environ.get("ATTN_MM_MODE", "fp32r")


def _mm_dt(mode):
    """Storage dtype for matmul operand tensors. float32r tensors must be
    written by a compute instruction (DVE/ACT) that performs the rounding —
    the BIR verifier enforces this provenance."""
    return f32r if mode == "fp32r" else f32


def _mc(ap, mode):
    # matmul operand dtype now lives on the tensor; kept for call-site compat
    return ap


def build_body(tc, x_d, qkvw_d, qkvb_d, projw_d, projb_d, out_d, mode, dbg=None,
               phases="all"):
    nc = tc.nc
    Act = mybir.ActivationFunctionType

    dm = _mm_dt(mode)
    with tc.tile_pool(name="persist", bufs=1) as persist:
        # ---- persistent tensors ----
        qkT_s = persist.tile([P, 2 * CK, N], dm)        # q,k features x tokens
        vnat_s = persist.tile([P, NT, (H // 2) * PW], dm)  # v pair blocks
        ident = persist.tile([P, P], f32)
        qkvb_qk = persist.tile([P, 2 * CK], f32)
        vb_bc = persist.tile([P, H, HD], f32)
        pb_bc = persist.tile([P, C], f32)

        make_identity(nc, ident)
        nc.sync.dma_start(qkvb_qk, qkvb_d[: 2 * C].rearrange("(m p) -> p m", p=P))
        nc.sync.dma_start(
            vb_bc, qkvb_d[2 * C :].rearrange("(h j) -> h j", j=HD).partition_broadcast(P)
        )
        nc.sync.dma_start(pb_bc, projb_d.partition_broadcast(P))

        # ones + zero filler columns of the v pair blocks (written once).
        # memset cannot write float32r; memset f32 then DVE-copy (rounds).
        vnat_w = vnat_s.rearrange("p t (j w) -> p t j w", w=PW)
        ones_f = persist.tile([P, 1], f32)
        zero_f = persist.tile([P, 1], f32)
        nc.vector.memset(ones_f, 1.0)
        nc.vector.memset(zero_f, 0.0)
        ones_row = persist.tile([P, P], dm)   # all-ones, lhsT of bcast matmuls
        nc.vector.tensor_copy(ones_row, ones_f.to_broadcast([P, P]))
        nc.vector.tensor_copy(
            vnat_w[:, :, :, HD : HD + 2],
            ones_f[:, None, None, :].to_broadcast([P, NT, H // 2, 2]),
        )
        nc.vector.tensor_copy(
            vnat_w[:, :, :, HD + 2 : VB_OFS],
            zero_f[:, None, None, :].to_broadcast([P, NT, H // 2, VB_OFS - HD - 2]),
        )

        # ================= phase A: load + transpose + qkv =================
        with (
            tc.tile_pool(name="phase_a", bufs=1) as pa,
            tc.tile_pool(name="xa", bufs=2) as xa,
            tc.tile_pool(name="pst", bufs=4, space="PSUM") as pst,
            tc.tile_pool(name="mmq", bufs=3, space="PSUM") as mmq,
        ):
            # DMA engine rotation: each engine owns its own DGE queues, so
            # spreading large loads across engines parallelizes the transfers
            dma_engs = [nc.sync, nc.gpsimd, nc.scalar]
            wq_s = pa.tile([P, CK, C3], dm)
            wq_src = qkvw_d.rearrange("(c p) n -> p c n", p=P)
            xT_s = pa.tile([P, CK, N], dm)

            x_r = x_d.rearrange("(t p) c -> t p c", p=P)
            for t in range(NT):
                x_t = xa.tile([P, C], f32, tag="xt")
                dma_engs[t % 3].dma_start(x_t, x_r[t])
                for c in range(CK):
                    pt = pst.tile([P, P], f32, tag="pt")
                    nc.tensor.transpose(pt, x_t[:, c * P : (c + 1) * P], ident)
                    nc.vector.tensor_copy(xT_s[:, c, t * P : (t + 1) * P], pt)
            for c in range(CK):
                dma_engs[c % 3].dma_start(wq_s[:, c], wq_src[:, c])

            if phases == "dma":
                # DMA-only bisect: skip all compute, just write something out
                out_r0 = out_d.rearrange("(t p) c -> t p c", p=P)
                for t in range(NT):
                    nc.sync.dma_start(out_r0[t], xT_s[:, :, t * P : (t + 1) * P].bitcast(f32))
                return

            # q,k features -> qkT  (feature on partitions)
            for m in range(2 * CK):
                for q2 in range(NQ):
                    ps = mmq.tile([P, QC], f32, tag="mm")
                    for k in range(CK):
                        nc.tensor.matmul(
                            ps,
                            lhsT=_mc(wq_s[:, k, m * P : (m + 1) * P], mode),
                            rhs=_mc(xT_s[:, k, q2 * QC : (q2 + 1) * QC], mode),
                            start=(k == 0),
                            stop=(k == CK - 1),
                        )
                    nc.vector.tensor_scalar_add(
                        out=qkT_s[:, m, q2 * QC : (q2 + 1) * QC],
                        in0=ps,
                        scalar1=qkvb_qk[:, m : m + 1],
                    )

            if dbg is not None:
                nc.sync.dma_start(dbg["xT"], xT_s)
                nc.sync.dma_start(dbg["qkT"], qkT_s)

            # v features -> vnat (token on partitions), strided per-head + bias
            for t in range(NT):
                for nv in range(2):
                    nsz = min(QC, C - nv * QC)  # 512, 256
                    h0, nh = nv * 8, nsz // HD
                    ps = mmq.tile([P, QC], f32, tag="mm")
                    for k in range(CK):
                        nc.tensor.matmul(
                            ps[:, :nsz],
                            lhsT=_mc(xT_s[:, k, t * P : (t + 1) * P], mode),
                            rhs=_mc(wq_s[:, k, 2 * C + nv * QC : 2 * C + nv * QC + nsz], mode),
                            start=(k == 0),
                            stop=(k == CK - 1),
                        )
                    pv = ps[:, :nsz].rearrange("p (h j) -> p h j", j=HD)
                    j0 = h0 // 2
                    nc.vector.tensor_add(
                        out=vnat_w[:, t, j0 : j0 + nh // 2, 0:HD],
                        in0=pv[:, 0::2],
                        in1=vb_bc[:, h0 : h0 + nh : 2, :],
                    )
                    nc.vector.tensor_add(
                        out=vnat_w[:, t, j0 : j0 + nh // 2, VB_OFS : VB_OFS + HD],
                        in0=pv[:, 1::2],
                        in1=vb_bc[:, h0 + 1 : h0 + nh : 2, :],
                    )

        if dbg is not None:
            nc.sync.dma_start(dbg["vnat"], vnat_s)

        if phases == "qkv":
            out_r0 = out_d.rearrange("(t p) c -> t p c", p=P)
            for t in range(NT):
                nc.sync.dma_start(out_r0[t], qkT_s[:, 0:CK, t * P : (t + 1) * P].bitcast(f32))
            return

        # ================= phase B: attention =================
        # separate pool so it reuses the space freed by phase A
        pbc_cm = tc.tile_pool(name="phase_bc", bufs=1)
        pbc = pbc_cm.__enter__()
        concatT_s = pbc.tile([P, CK, N], dm)        # normalized attn out^T
        wp_s = pbc.tile([P, CK, C], dm)
        wp_src = projw_d.rearrange("(c p) n -> p c n", p=P)
        dma_engs2 = [nc.sync, nc.gpsimd, nc.scalar]
        for c in range(CK):
            dma_engs2[c % 3].dma_start(wp_s[:, c], wp_src[:, c])

        with (
            tc.tile_pool(name="exps", bufs=1) as exps,
            tc.tile_pool(name="rpool", bufs=2) as rpool,
            tc.tile_pool(name="sc", bufs=2, space="PSUM") as sc,
            tc.tile_pool(name="avp", bufs=1, space="PSUM") as avp,
        ):
            for j in range(H // 2):
                for q2 in range(NQ):
                    qs = slice(q2 * QC, (q2 + 1) * QC)
                    expA = exps.tile([P, NT, QC], dm, tag="expA")
                    expB = exps.tile([P, NT, QC], dm, tag="expB")
                    # --- scores + exp, two k-tiles per ACT call ---
                    for kp in range(NT // 2):
                        psA = sc.tile([P, 2, QC], f32, tag="sc")
                        psB = sc.tile([P, 2, QC], f32, tag="sc")
                        for u in range(2):
                            kt = 2 * kp + u
                            ks = slice(kt * P, (kt + 1) * P)
                            nc.tensor.matmul(
                                psA[:, u],
                                lhsT=_mc(qkT_s[0:HD, CK + j, ks], mode),
                                rhs=_mc(qkT_s[0:HD, j, qs], mode),
                                start=True, stop=True,
                            )
                            nc.tensor.matmul(
                                psB[:, u],
                                lhsT=_mc(qkT_s[HD:P, CK + j, ks], mode),
                                rhs=_mc(qkT_s[HD:P, j, qs], mode),
                                start=True, stop=True,
                            )
                        nc.scalar.activation(
                            expA[:, 2 * kp : 2 * kp + 2, :], psA, Act.Exp, scale=0.125
                        )
                        nc.scalar.activation(
                            expB[:, 2 * kp : 2 * kp + 2, :], psB, Act.Exp, scale=0.125
                        )
                    if dbg is not None and j == 0 and q2 == 0:
                        nc.sync.dma_start(dbg["expA"], expA)
                        nc.sync.dma_start(dbg["expB"], expB)
                    # --- AV + denominator sums (fused via the pair-block
                    # lhsT views), accumulated over k tiles ---
                    psAV_A = avp.tile([P, QC], f32, tag="avA")
                    psAV_B = avp.tile([P, QC], f32, tag="avB")
                    for kt in range(NT):
                        st, sp = kt == 0, kt == NT - 1
                        lA = vnat_s[:, kt, j * PW : j * PW + P]
                        lB = vnat_s[:, kt, j * PW + OFS_B : j * PW + OFS_B + P]
                        nc.tensor.matmul(
                            psAV_A, lhsT=lA, rhs=_mc(expA[:, kt], mode),
                            start=st, stop=sp,
                        )
                        nc.tensor.matmul(
                            psAV_B, lhsT=lB, rhs=_mc(expB[:, kt], mode),
                            start=st, stop=sp,
                        )
                    # --- normalize into concatT (A sums at psAV_A[64],
                    # B sums at psAV_B[32]) ---
                    # reciprocal (DVE, written as f32r so it can feed a matmul),
                    # then partition-broadcast via PE ones outer product.
                    r_ab = rpool.tile([65, QC], dm, tag="rab")
                    with nc.allow_low_precision(reason="f32r is 4-byte; rounding only"):
                        nc.vector.reciprocal(r_ab[64:65], psAV_A[HD : HD + 1])
                        nc.vector.reciprocal(r_ab[32:33], psAV_B[32:33])
                    psR_A = avp.tile([P, QC], f32, tag="psRA")
                    psR_B = avp.tile([P, QC], f32, tag="psRB")
                    nc.tensor.matmul(
                        psR_A, lhsT=ones_row[HD : HD + 1, :], rhs=r_ab[64:65, :],
                        start=True, stop=True,
                    )
                    nc.tensor.matmul(
                        psR_B, lhsT=ones_row[32:33, :], rhs=r_ab[32:33, :],
                        start=True, stop=True,
                    )
                    rbc = rpool.tile([P, 1, QC], f32, tag="rbc")
                    nc.vector.tensor_copy(rbc[0:HD, 0], psR_A[0:HD])
                    nc.vector.tensor_copy(rbc[HD:P, 0], psR_B[HD:P])
                    nc.vector.tensor_mul(
                        out=concatT_s[0:HD, j, qs], in0=psAV_A[0:HD], in1=rbc[0:HD, 0]
                    )
                    nc.vector.tensor_mul(
                        out=concatT_s[HD:P, j, qs], in0=psAV_B[HD:P], in1=rbc[HD:P, 0]
                    )

        if dbg is not None:
            nc.sync.dma_start(dbg["concatT"], concatT_s)

        if phases == "attn":
            out_r0 = out_d.rearrange("(t p) c -> t p c", p=P)
            for t in range(NT):
                nc.sync.dma_start(out_r0[t], concatT_s[:, :, t * P : (t + 1) * P].bitcast(f32))
            pbc_cm.__exit__(None, None, None)
            return

        # ================= phase C: output projection =================
        with (
            tc.tile_pool(name="outs", bufs=3) as outs,
            tc.tile_pool(name="mmp", bufs=3, space="PSUM") as mmp,
        ):
            out_r = out_d.rearrange("(t p) c -> t p c", p=P)
            for t in range(NT):
                out_t = outs.tile([P, C], f32, tag="ot")
                for n2 in range(2):
                    nsz = min(QC, C - n2 * QC)
                    ns = slice(n2 * QC, n2 * QC + nsz)
                    ps = mmp.tile([P, QC], f32, tag="mmp")
                    for c in range(CK):
                        nc.tensor.matmul(
                            ps[:, :nsz],
                            lhsT=_mc(concatT_s[:, c, t * P : (t + 1) * P], mode),
                            rhs=_mc(wp_s[:, c, ns], mode),
                            start=(c == 0),
                            stop=(c == CK - 1),
                        )
                    nc.vector.tensor_add(out=out_t[:, ns], in0=ps[:, :nsz], in1=pb_bc[:, ns])
                [nc.sync, nc.gpsimd, nc.scalar][t % 3].dma_start(out_r[t], out_t)
        pbc_cm.__exit__(None, None, None)


def build(mode=MODE, repeat=1, debug_dumps=False, phases="all"):
    nc = bacc.Bacc(
        "TRN2",
        target_bir_lowering=False,
        debug=False,
        enable_asserts=False,
        num_devices=B,
    )
    dmw = _mm_dt(mode)
    x_d = nc.dram_tensor("x", [N, C], f32, kind="ExternalInput").ap()
    qkvw_d = nc.dram_tensor("qkv_w", [C, C3], dmw, kind="ExternalInput").ap()
    qkvb_d = nc.dram_tensor("qkv_b", [C3], f32, kind="ExternalInput").ap()
    projw_d = nc.dram_tensor("proj_w", [C, C], dmw, kind="ExternalInput").ap()
    projb_d = nc.dram_tensor("proj_b", [C], f32, kind="ExternalInput").ap()
    out_d = nc.dram_tensor("out", [N, C], f32, kind="ExternalOutput").ap()

    dbg = None
    if debug_dumps:
        dbg = {
            "xT": nc.dram_tensor("dbg_xT", [P, CK, N], f32, kind="ExternalOutput").ap(),
            "qkT": nc.dram_tensor("dbg_qkT", [P, 2 * CK, N], f32, kind="ExternalOutput").ap(),
            "vnat": nc.dram_tensor("dbg_vnat", [P, NT, (H // 2) * PW], f32, kind="ExternalOutput").ap(),
            "expA": nc.dram_tensor("dbg_expA", [P, NT, QC], f32, kind="ExternalOutput").ap(),
            "expB": nc.dram_tensor("dbg_expB", [P, NT, QC], f32, kind="ExternalOutput").ap(),
            "rbc": nc.dram_tensor("dbg_rbc", [P, 1, QC], f32, kind="ExternalOutput").ap(),
            "concatT": nc.dram_tensor("dbg_concatT", [P, CK, N], f32, kind="ExternalOutput").ap(),
        }

    with tile.TileContext(nc) as tc:
        if repeat == 1:
            build_body(tc, x_d, qkvw_d, qkvb_d, projw_d, projb_d, out_d, mode, dbg=dbg, phases=phases)
        else:
            # hardware loop: constant NEFF size, repeat bodies back-to-back --
            # used for timing (wall-clock differencing between repeat counts)
            with tc.For_i(
                0, repeat, 1,
                hint_engines=(mybir.EngineType.PE, mybir.EngineType.DVE),
            ):
                build_body(tc, x_d, qkvw_d, qkvb_d, projw_d, projb_d, out_d, mode, dbg=dbg, phases=phases)
    nc.compile()
    return nc


_NC_CACHE = {}


def _get_nc(mode, repeat=1):
    key = (mode, repeat)
    if key not in _NC_CACHE:
        _NC_CACHE[key] = build(mode, repeat)
    return _NC_CACHE[key]


def kernel(x, qkv_w, qkv_b, proj_w, proj_b):
    x = np.asarray(x, dtype=np.float32)
    qkv_w = np.asarray(qkv_w, dtype=np.float32)
    qkv_b = np.asarray(qkv_b, dtype=np.float32)
    proj_w = np.asarray(proj_w, dtype=np.float32)
    proj_b = np.asarray(proj_b, dtype=np.float32)

    nc = _get_nc(MODE, 1)
    in_maps = [
        {
            "x": np.ascontiguousarray(x[b]),
            "qkv_w": qkv_w,
            "qkv_b": qkv_b,
            "proj_w": proj_w,
            "proj_b": proj_b,
        }
        for b in range(B)
    ]
    res = run_bass_kernel_spmd(nc, in_maps, core_ids=list(range(B)))
    return np.stack([res.results[b]["out"] for b in range(B)]).astype(np.float32)


# revision 38
# speedup vs baseline: 3.6682x; 1.7354x over previous
"""Multi-head attention (B=8, N=1024, C=768, H=12) on 8 TRN2 NeuronCores.

Strategy: pure data parallelism over the batch dim — each core computes one
batch element's full attention block. Weights are replicated; no collectives.

Matmuls run in float32r (full-rate TF32-like PE mode, ~1.7e-4 per-matmul
rel err vs 4x-slower exact fp32; select with ATTN_MM_MODE=fp32).

Per-core pipeline (f32r storage for all matmul operands):
  1. x [1024,768] -> PE transpose -> xT [768,1024] in SBUF
  2. qkv(q,k):  qkT[feature, tok] = (qkv_w chunk).T @ xT            (PE)
     qkv(v) -> vnat pair blocks [vA|onesA|onesB|zeros|vB] per head pair
               (tok on partitions; the ones columns become the softmax
                denominator rows of the AV matmul)
  3. per head pair (A,B live in SBUF partition halves 0:64 / 64:128):
     scoresT[ktok, q] = kT.T @ qT  (two row-tiled concurrent matmuls,
                                    hd=64 contraction at rows 0/64)
     expT = Exp(scoresT * 0.125)   (ACT reads PSUM, writes f32r SBUF)
     AV:  psAV_A += block[0:128].T  @ expA   rows: 0-63 out, 64 sums
          psAV_B += block[33:161].T @ expB   rows: 32 sums, 64-127 out
     (fp32r matmuls require dst partition 0, so sums ride inside the
      M=128 AV matmul via the interleaved ones columns)
     normalize: r = 1/sums (DVE), partition-broadcast via PE ones outer
     product, concatT[64h+hd, tok] = psAV * r  (DVE)
  4. proj: out[tok, c] = concatT_chunk.T @ proj_w + proj_b  (PE + DVE)

Large DMAs are spread round-robin over the three DMA-capable engine
queues (SP, GpSimd, ACT) — single-queue issue is ~2x slower end-to-end.

Timing methodology (test.py): the body is wrapped in a hardware For_i
loop; per-iteration time = (wall(rep=514) - wall(rep=2)) / 512, which
cancels the ~2s axon-tunnel call overhead.
"""

import os
import numpy as np

import concourse.bass as bass
import concourse.tile as tile
from concourse import bacc, mybir
from concourse.bass_utils import run_bass_kernel_spmd
from concourse.masks import make_identity

B, N, C, H, HD = 8, 1024, 768, 12, 64
C3 = 3 * C
P = 128
NT = N // P   # 8 token tiles
CK = C // P   # 6 C chunks
QC = 512      # moving-operand chunk (fp32 max 512)
NQ = N // QC  # 2
f32 = mybir.dt.float32
f32r = mybir.dt.float32r

# v pair-block layout: per head pair j the columns are
#   [ vA(0:64) | onesA(64) | onesB(65) | zeros(66:97) | vB(97:161) ]
# lhsT_A = block[0:128]   -> psum rows: 0-63 A-out, 64 A-sums
# lhsT_B = block[33:161]  -> psum rows: 32 B-sums, 64-127 B-out
# Both views are M=128 matmuls with dst partition 0 (required by fp32r),
# and the sums land on 32-aligned psum rows for DVE access.
PW = 161       # pair block width
OFS_B = 33     # lhsT_B offset within the block
VB_OFS = 97    # vB column offset

# matmul operand dtype mode: "fp32" (exact, 4 cyc/row) or "fp32r" (1 cyc/row)
MODE = os.environ.get("ATTN_MM_MODE", "fp32r")


def _mm_dt(mode):
    """Storage dtype for matmul operand tensors. float32r tensors must be
    written by a compute instruction (DVE/ACT) that performs the rounding —
    the BIR verifier enforces this provenance."""
    return f32r if mode == "fp32r" else f32


def _mc(ap, mode):
    # matmul operand dtype now lives on the tensor; kept for call-site compat
    return ap


def build_body(tc, x_d, qkvw_d, qkvb_d, projw_d, projb_d, out_d, mode, dbg=None,
               phases="all"):
    nc = tc.nc
    Act = mybir.ActivationFunctionType

    dm = _mm_dt(mode)
    with tc.tile_pool(name="persist", bufs=1) as persist:
        # ---- persistent tensors ----
        qkT_s = persist.tile([P, 2 * CK, N], dm)        # q,k features x tokens
        vnat_s = persist.tile([P, NT, (H // 2) * PW], dm)  # v pair blocks
        ident = persist.tile([P, P], f32)
        qkvb_qk = persist.tile([P, 2 * CK], f32)
        vb_bc = persist.tile([P, H, HD], f32)
        pb_bc = persist.tile([P, C], f32)

        make_identity(nc, ident)
        nc.sync.dma_start(qkvb_qk, qkvb_d[: 2 * C].rearrange("(m p) -> p m", p=P))
        nc.sync.dma_start(
            vb_bc, qkvb_d[2 * C :].rearrange("(h j) -> h j", j=HD).partition_broadcast(P)
        )
        nc.sync.dma_start(pb_bc, projb_d.partition_broadcast(P))

        # ones + zero filler columns of the v pair blocks (written once).
        # memset cannot write float32r; memset f32 then DVE-copy (rounds).
        vnat_w = vnat_s.rearrange("p t (j w) -> p t j w", w=PW)
        ones_f = persist.tile([P, 1], f32)
        zero_f = persist.tile([P, 1], f32)
        nc.vector.memset(ones_f, 1.0)
        nc.vector.memset(zero_f, 0.0)
        ones_row = persist.tile([P, P], dm)   # all-ones, lhsT of bcast matmuls
        nc.vector.tensor_copy(ones_row, ones_f.to_broadcast([P, P]))
        nc.vector.tensor_copy(
            vnat_w[:, :, :, HD : HD + 2],
            ones_f[:, None, None, :].to_broadcast([P, NT, H // 2, 2]),
        )
        nc.vector.tensor_copy(
            vnat_w[:, :, :, HD + 2 : VB_OFS],
            zero_f[:, None, None, :].to_broadcast([P, NT, H // 2, VB_OFS - HD - 2]),
        )

        # ================= phase A: load + transpose + qkv =================
        with (
            tc.tile_pool(name="phase_a", bufs=1) as pa,
            tc.tile_pool(name="xa", bufs=2) as xa,
            tc.tile_pool(name="pst", bufs=4, space="PSUM") as pst,
            tc.tile_pool(name="mmq", bufs=3, space="PSUM") as mmq,
        ):
            # DMA engine rotation: each engine owns its own DGE queues, so
            # spreading large loads across engines parallelizes the transfers
            dma_engs = [nc.sync, nc.gpsimd, nc.scalar]
            wq_s = pa.tile([P, CK, C3], dm)
            wq_src = qkvw_d.rearrange("(c p) n -> p c n", p=P)
            xT_s = pa.tile([P, CK, N], dm)

            x_r = x_d.rearrange("(t p) c -> t p c", p=P)
            for t in range(NT):
                x_t = xa.tile([P, C], f32, tag="xt")
                dma_engs[t % 3].dma_start(x_t, x_r[t])
                for c in range(CK):
                    pt = pst.tile([P, P], f32, tag="pt")
                    nc.tensor.transpose(pt, x_t[:, c * P : (c + 1) * P], ident)
                    nc.vector.tensor_copy(xT_s[:, c, t * P : (t + 1) * P], pt)
            for c in range(CK):
                dma_engs[c % 3].dma_start(wq_s[:, c], wq_src[:, c])

            if phases == "dma":
                # DMA-only bisect: skip all compute, just write something out
                out_r0 = out_d.rearrange("(t p) c -> t p c", p=P)
                for t in range(NT):
                    nc.sync.dma_start(out_r0[t], xT_s[:, :, t * P : (t + 1) * P].bitcast(f32))
                return

            # q,k features -> qkT  (feature on partitions)
            for m in range(2 * CK):
                for q2 in range(NQ):
                    ps = mmq.tile([P, QC], f32, tag="mm")
                    for k in range(CK):
                        nc.tensor.matmul(
                            ps,
                            lhsT=_mc(wq_s[:, k, m * P : (m + 1) * P], mode),
                            rhs=_mc(xT_s[:, k, q2 * QC : (q2 + 1) * QC], mode),
                            start=(k == 0),
                            stop=(k == CK - 1),
                        )
                    nc.vector.tensor_scalar_add(
                        out=qkT_s[:, m, q2 * QC : (q2 + 1) * QC],
                        in0=ps,
                        scalar1=qkvb_qk[:, m : m + 1],
                    )

            if dbg is not None:
                nc.sync.dma_start(dbg["xT"], xT_s)
                nc.sync.dma_start(dbg["qkT"], qkT_s)

            # v features -> vnat (token on partitions), strided per-head + bias
            for t in range(NT):
                for nv in range(2):
                    nsz = min(QC, C - nv * QC)  # 512, 256
                    h0, nh = nv * 8, nsz // HD
                    ps = mmq.tile([P, QC], f32, tag="mm")
                    for k in range(CK):
                        nc.tensor.matmul(
                            ps[:, :nsz],
                            lhsT=_mc(xT_s[:, k, t * P : (t + 1) * P], mode),
                            rhs=_mc(wq_s[:, k, 2 * C + nv * QC : 2 * C + nv * QC + nsz], mode),
                            start=(k == 0),
                            stop=(k == CK - 1),
                        )
                    pv = ps[:, :nsz].rearrange("p (h j) -> p h j", j=HD)
                    j0 = h0 // 2
                    nc.vector.tensor_add(
                        out=vnat_w[:, t, j0 : j0 + nh // 2, 0:HD],
                        in0=pv[:, 0::2],
                        in1=vb_bc[:, h0 : h0 + nh : 2, :],
                    )
                    nc.vector.tensor_add(
                        out=vnat_w[:, t, j0 : j0 + nh // 2, VB_OFS : VB_OFS + HD],
                        in0=pv[:, 1::2],
                        in1=vb_bc[:, h0 + 1 : h0 + nh : 2, :],
                    )

        if dbg is not None:
            nc.sync.dma_start(dbg["vnat"], vnat_s)

        if phases == "qkv":
            out_r0 = out_d.rearrange("(t p) c -> t p c", p=P)
            for t in range(NT):
                nc.sync.dma_start(out_r0[t], qkT_s[:, 0:CK, t * P : (t + 1) * P].bitcast(f32))
            return

        # ================= phase B: attention =================
        # separate pool so it reuses the space freed by phase A
        pbc_cm = tc.tile_pool(name="phase_bc", bufs=1)
        pbc = pbc_cm.__enter__()
        concatT_s = pbc.tile([P, CK, N], dm)        # normalized attn out^T
        wp_s = pbc.tile([P, CK, C], dm)
        wp_src = projw_d.rearrange("(c p) n -> p c n", p=P)
        dma_engs2 = [nc.sync, nc.gpsimd, nc.scalar]
        for c in range(CK):
            dma_engs2[c % 3].dma_start(wp_s[:, c], wp_src[:, c])

        with (
            tc.tile_pool(name="exps", bufs=1) as exps,
            tc.tile_pool(name="rpool", bufs=2) as rpool,
            tc.tile_pool(name="sc", bufs=2, space="PSUM") as sc,
            tc.tile_pool(name="avp", bufs=1, space="PSUM") as avp,
        ):
            for j in range(H // 2):
                for q2 in range(NQ):
                    qs = slice(q2 * QC, (q2 + 1) * QC)
                    expA = exps.tile([P, NT, QC], dm, tag="expA")
                    expB = exps.tile([P, NT, QC], dm, tag="expB")
                    # --- scores + exp, two k-tiles per ACT call ---
                    for kp in range(NT // 2):
                        psA = sc.tile([P, 2, QC], f32, tag="sc")
                        psB = sc.tile([P, 2, QC], f32, tag="sc")
                        for u in range(2):
                            kt = 2 * kp + u
                            ks = slice(kt * P, (kt + 1) * P)
                            nc.tensor.matmul(
                                psA[:, u],
                                lhsT=_mc(qkT_s[0:HD, CK + j, ks], mode),
                                rhs=_mc(qkT_s[0:HD, j, qs], mode),
                                start=True, stop=True,
                            )
                            nc.tensor.matmul(
                                psB[:, u],
                                lhsT=_mc(qkT_s[HD:P, CK + j, ks], mode),
                                rhs=_mc(qkT_s[HD:P, j, qs], mode),
                                start=True, stop=True,
                            )
                        nc.scalar.activation(
                            expA[:, 2 * kp : 2 * kp + 2, :], psA, Act.Exp, scale=0.125
                        )
                        nc.scalar.activation(
                            expB[:, 2 * kp : 2 * kp + 2, :], psB, Act.Exp, scale=0.125
                        )
                    if dbg is not None and j == 0 and q2 == 0:
                        nc.sync.dma_start(dbg["expA"], expA)
                        nc.sync.dma_start(dbg["expB"], expB)
                    # --- AV + denominator sums (fused via the pair-block
                    # lhsT views), accumulated over k tiles ---
                    psAV_A = avp.tile([P, QC], f32, tag="avA")
                    psAV_B = avp.tile([P, QC], f32, tag="avB")
                    for kt in range(NT):
                        st, sp = kt == 0, kt == NT - 1
                        lA = vnat_s[:, kt, j * PW : j * PW + P]
                        lB = vnat_s[:, kt, j * PW + OFS_B : j * PW + OFS_B + P]
                        nc.tensor.matmul(
                            psAV_A, lhsT=lA, rhs=_mc(expA[:, kt], mode),
                            start=st, stop=sp,
                        )
                        nc.tensor.matmul(
                            psAV_B, lhsT=lB, rhs=_mc(expB[:, kt], mode),
                            start=st, stop=sp,
                        )
                    # --- normalize into concatT (A sums at psAV_A[64],
                    # B sums at psAV_B[32]) ---
                    # reciprocal (DVE, written as f32r so it can feed a matmul),
                    # then partition-broadcast via PE ones outer product.
                    r_ab = rpool.tile([65, QC], dm, tag="rab")
                    with nc.allow_low_precision(reason="f32r is 4-byte; rounding only"):
                        nc.vector.reciprocal(r_ab[64:65], psAV_A[HD : HD + 1])
                        nc.vector.reciprocal(r_ab[32:33], psAV_B[32:33])
                    psR_A = avp.tile([P, QC], f32, tag="psRA")
                    psR_B = avp.tile([P, QC], f32, tag="psRB")
                    nc.tensor.matmul(
                        psR_A, lhsT=ones_row[HD : HD + 1, :], rhs=r_ab[64:65, :],
                        start=True, stop=True,
                    )
                    nc.tensor.matmul(
                        psR_B, lhsT=ones_row[32:33, :], rhs=r_ab[32:33, :],
                        start=True, stop=True,
                    )
                    rbc = rpool.tile([P, 1, QC], f32, tag="rbc")
                    nc.vector.tensor_copy(rbc[0:HD, 0], psR_A[0:HD])
                    nc.vector.tensor_copy(rbc[HD:P, 0], psR_B[HD:P])
                    nc.vector.tensor_mul(
                        out=concatT_s[0:HD, j, qs], in0=psAV_A[0:HD], in1=rbc[0:HD, 0]
                    )
                    nc.vector.tensor_mul(
                        out=concatT_s[HD:P, j, qs], in0=psAV_B[HD:P], in1=rbc[HD:P, 0]
                    )

        if dbg is not None:
            nc.sync.dma_start(dbg["concatT"], concatT_s)

        if phases == "attn":
            out_r0 = out_d.rearrange("(t p) c -> t p c", p=P)
            for t in range(NT):
                nc.sync.dma_start(out_r0[t], concatT_s[:, :, t * P : (t + 1) * P].bitcast(f32))
            pbc_cm.__exit__(None, None, None)
            return

        # ================= phase C: output projection =================
        with (
            tc.tile_pool(name="outs", bufs=3) as outs,
            tc.tile_pool(name="mmp", bufs=3, space="PSUM") as mmp,
        ):
            out_r = out_d.rearrange("(t p) c -> t p c", p=P)
            for t in range(NT):
                out_t = outs.tile([P, C], f32, tag="ot")
                for n2 in range(2):
                    nsz = min(QC, C - n2 * QC)
                    ns = slice(n2 * QC, n2 * QC + nsz)
                    ps = mmp.tile([P, QC], f32, tag="mmp")
                    for c in range(CK):
                        nc.tensor.matmul(
                            ps[:, :nsz],
                            lhsT=_mc(concatT_s[:, c, t * P : (t + 1) * P], mode),
                            rhs=_mc(wp_s[:, c, ns], mode),
                            start=(c == 0),
                            stop=(c == CK - 1),
                        )
                    nc.vector.tensor_add(out=out_t[:, ns], in0=ps[:, :nsz], in1=pb_bc[:, ns])
                [nc.sync, nc.gpsimd, nc.scalar][t % 3].dma_start(out_r[t], out_t)
        pbc_cm.__exit__(None, None, None)


def build(mode=MODE, repeat=1, debug_dumps=False, phases="all"):
    nc = bacc.Bacc(
        "TRN2",
        target_bir_lowering=False,
        debug=False,
        enable_asserts=False,
        num_devices=B,
    )
    dmw = _mm_dt(mode)
    x_d = nc.dram_tensor("x", [N, C], f32, kind="ExternalInput").ap()
    qkvw_d = nc.dram_tensor("qkv_w", [C, C3], dmw, kind="ExternalInput").ap()
    qkvb_d = nc.dram_tensor("qkv_b", [C3], f32, kind="ExternalInput").ap()
    projw_d = nc.dram_tensor("proj_w", [C, C], dmw, kind="ExternalInput").ap()
    projb_d = nc.dram_tensor("proj_b", [C], f32, kind="ExternalInput").ap()
    out_d = nc.dram_tensor("out", [N, C], f32, kind="ExternalOutput").ap()

    dbg = None
    if debug_dumps:
        dbg = {
            "xT": nc.dram_tensor("dbg_xT", [P, CK, N], f32, kind="ExternalOutput").ap(),
            "qkT": nc.dram_tensor("dbg_qkT", [P, 2 * CK, N], f32, kind="ExternalOutput").ap(),
            "vnat": nc.dram_tensor("dbg_vnat", [P, NT, (H // 2) * PW], f32, kind="ExternalOutput").ap(),
            "expA": nc.dram_tensor("dbg_expA", [P, NT, QC], f32, kind="ExternalOutput").ap(),
            "expB": nc.dram_tensor("dbg_expB", [P, NT, QC], f32, kind="ExternalOutput").ap(),
            "rbc": nc.dram_tensor("dbg_rbc", [P, 1, QC], f32, kind="ExternalOutput").ap(),
            "concatT": nc.dram_tensor("dbg_concatT", [P, CK, N], f32, kind="ExternalOutput").ap(),
        }

    with tile.TileContext(nc) as tc:
        if repeat == 1:
            build_body(tc, x_d, qkvw_d, qkvb_d, projw_d, projb_d, out_d, mode, dbg=dbg, phases=phases)
        else:
            # hardware loop: constant NEFF size, repeat bodies back-to-back --
            # used for timing (wall-clock differencing between repeat counts)
            with tc.For_i(
                0, repeat, 1,
                hint_engines=(mybir.EngineType.PE, mybir.EngineType.DVE),
            ):
                build_body(tc, x_d, qkvw_d, qkvb_d, projw_d, projb_d, out_d, mode, dbg=dbg, phases=phases)
    nc.compile()
    return nc


_NC_CACHE = {}


def _get_nc(mode, repeat=1):
    key = (mode, repeat)
    if key not in _NC_CACHE:
        _NC_CACHE[key] = build(mode, repeat)
    return _NC_CACHE[key]


def kernel(x, qkv_w, qkv_b, proj_w, proj_b):
    x = np.asarray(x, dtype=np.float32)
    qkv_w = np.asarray(qkv_w, dtype=np.float32)
    qkv_b = np.asarray(qkv_b, dtype=np.float32)
    proj_w = np.asarray(proj_w, dtype=np.float32)
    proj_b = np.asarray(proj_b, dtype=np.float32)

    nc = _get_nc(MODE, 1)
    in_maps = [
        {
            "x": np.ascontiguousarray(x[b]),
            "qkv_w": qkv_w,
            "qkv_b": qkv_b,
            "proj_w": proj_w,
            "proj_b": proj_b,
        }
        for b in range(B)
    ]
    res = run_bass_kernel_spmd(nc, in_maps, core_ids=list(range(B)))
    return np.stack([res.results[b]["out"] for b in range(B)]).astype(np.float32)
